# revision 24
# baseline (speedup 1.0000x reference)
"""Trainium2 Bass kernel for the AttentionBranch conv-BN-ReLU pipeline.

Pipeline (reference):
    conva: 3x3 conv (2048->512, pad 1, no bias) -> sync-BN -> ReLU
    convb: 1x1 conv (512->512, bias)
    concat([x, convb_out]) -> conv1: 3x3 conv (2560->512, pad 1, no bias)
        -> sync-BN -> ReLU  (= out)
    conv4: 1x1 conv (512->19, bias)  (= out_final)

Sharding: data-parallel over (batch=4) x (H halves=2) = 8 shards, one per
NeuronCore. Each core computes 34 conv-a rows (its 32 owned rows + 1 halo row
each side; rows outside the image are masked) so that conv1's 3x3 stencil
never needs activations from another core. BatchNorm statistics are made
global with two tiny (128x8 f32) AllReduces.

Convs are computed on the PE as accumulations over (c_tile, ki, kj) shifted
matmuls against a zero-padded input resident in SBUF. All matmul inputs are
bf16, accumulation in fp32 PSUM.
"""

import os

import numpy as np
import ml_dtypes

from concourse import bacc, tile, mybir
import concourse.bass as bass
from concourse.bass_utils import run_bass_kernel_spmd

BF16 = mybir.dt.bfloat16
F32 = mybir.dt.float32
AF = mybir.ActivationFunctionType
ALU = mybir.AluOpType

# Problem shape (hardcoded; kernel.py must be self-contained)
B, CIN, CMID, NCLS, H, W = 4, 2048, 512, 19, 64, 64
NCORES = 8
EPS = 1e-5
COUNT = B * H * W  # BN reduction count (global)

# Per-core spatial shard: 32 owned rows, 34 computed conv-a rows,
# padded x buffer of 36 rows x 66 cols.
OWN = 32          # owned output rows per core
AR = 34           # conv-a rows computed (owned + 1 halo each side)
XR, XC = 36, 66   # padded x shard rows/cols
CT_X = CIN // 128         # 16 c-tiles of x
CT_A = CMID // 128        # 4 c-tiles of mid activations
MT = CMID // 128          # 4 m-tiles of conv outputs
STEPS_A = CT_X * 9        # 144 accumulation steps for conva
STEPS_1 = (CT_X + CT_A) * 9  # 180 steps for conv1
CHUNK = 36                # weight-chunk steps per DMA
# conva n-tiles over 34 rows: 4 full (8 rows) + 1 partial (2 rows)
NT_A = [(0, 8), (8, 8), (16, 8), (24, 8), (32, 2)]
# conv1/conv4 n-tiles over 32 rows
NT_1 = [(0, 8), (8, 8), (16, 8), (24, 8)]


def _decode(step):
    return step // 9, (step % 9) // 3, step % 3  # (c_tile, ki, kj)


# Debug bisect: 1=conva, 2=+bn1, 3=+convb, 4=+conv1, 5=+bn2, 99=full
KSTAGE = int(os.environ.get("KSTAGE", "99"))


def _build():
    nc = bacc.Bacc(
        "TRN2",
        target_bir_lowering=False,
        debug=False,
        enable_asserts=False,
        num_devices=NCORES,
    )

    # ---- I/O tensors (per-core) ----
    xp_d = nc.dram_tensor("xp", [CT_X, 128, XR, XC], BF16, kind="ExternalInput")
    mask_d = nc.dram_tensor("mask", [128, AR, W], BF16, kind="ExternalInput")
    wa_d = nc.dram_tensor("wa", [MT, 4, 128, CHUNK, 128], BF16, kind="ExternalInput")
    w1_d = nc.dram_tensor("w1", [MT, 5, 128, CHUNK, 128], BF16, kind="ExternalInput")
    wb_d = nc.dram_tensor("wb", [128, CT_A, MT, 128], BF16, kind="ExternalInput")
    w4_d = nc.dram_tensor("w4", [128, CT_A, NCLS], BF16, kind="ExternalInput")
    ga_d = nc.dram_tensor("ga", [128, MT], F32, kind="ExternalInput")
    ba_d = nc.dram_tensor("ba", [128, MT], F32, kind="ExternalInput")
    bb_d = nc.dram_tensor("bb", [128, MT], F32, kind="ExternalInput")
    g1_d = nc.dram_tensor("g1", [128, MT], F32, kind="ExternalInput")
    b1_d = nc.dram_tensor("b1", [128, MT], F32, kind="ExternalInput")
    b4_d = nc.dram_tensor("b4", [128, 1], F32, kind="ExternalInput")

    out_d = nc.dram_tensor("out_sh", [MT, 128, OWN, W], F32, kind="ExternalOutput")
    ofin_d = nc.dram_tensor("ofin_sh", [NCLS, OWN, W], F32, kind="ExternalOutput")

    with tile.TileContext(nc) as tc:
        _emit(nc, tc, xp_d, mask_d, wa_d, w1_d, wb_d, w4_d,
              ga_d, ba_d, bb_d, g1_d, b1_d, b4_d, out_d, ofin_d)

    nc.compile()
    return nc


def _emit(nc, tc, xp_d, mask_d, wa_d, w1_d, wb_d, w4_d,
          ga_d, ba_d, bb_d, g1_d, b1_d, b4_d, out_d, ofin_d):
    with tc.tile_pool(name="persist", bufs=1) as P, \
         tc.tile_pool(name="wpool", bufs=2) as WP, \
         tc.tile_pool(name="ev", bufs=2) as EV, \
         tc.tile_pool(name="fout", bufs=2) as FO, \
         tc.tile_pool(name="psum", bufs=8, space="PSUM") as PS, \
         tc.tile_pool(name="dram", bufs=1, space="DRAM") as DR:

        # ---- persistent SBUF tensors + input DMAs ----
        xp = []
        for ct in range(CT_X):
            t = P.tile([128, XR, XC], BF16, tag=f"xp{ct}", name=f"xp{ct}")
            nc.sync.dma_start(t[:], xp_d.ap()[ct])
            xp.append(t)
        mask = P.tile([128, AR, W], BF16, tag="mask", name="mask")
        nc.sync.dma_start(mask[:], mask_d.ap())
        wbt = P.tile([128, CT_A, MT, 128], BF16, tag="wbt", name="wbt")
        nc.sync.dma_start(wbt[:], wb_d.ap())
        w4t = P.tile([128, CT_A, NCLS], BF16, tag="w4t", name="w4t")
        nc.sync.dma_start(w4t[:], w4_d.ap())

        def small(name, src=None, cols=MT):
            t = P.tile([128, cols], F32, tag=name, name=name)
            if src is not None:
                nc.sync.dma_start(t[:], src.ap())
            return t

        ga = small("ga", ga_d)
        ba = small("ba", ba_d)
        bb = small("bb", bb_d)
        g1 = small("g1", g1_d)
        b1 = small("b1", b1_d)
        b4 = small("b4", b4_d, cols=1)

        # out_b padded buffers (zero borders), conv-a sbuf, activations
        obp = []
        for m in range(MT):
            t = P.tile([128, AR, XC], BF16, tag=f"obp{m}", name=f"obp{m}")
            nc.vector.memset(t[:], 0)
            obp.append(t)
        csb = [P.tile([128, AR, W], BF16, tag=f"csb{m}", name=f"csb{m}")
               for m in range(MT)]
        act = [P.tile([128, AR, W], BF16, tag=f"act{m}", name=f"act{m}")
               for m in range(MT)]
        # conv1 output (bf16) reuses the conv-a slots (disjoint lifetime)
        c1sb = [P.tile([128, OWN, W], BF16, tag=f"csb{m}", name=f"c1sb{m}")
                for m in range(MT)]
        # bf16 copy of `out` for conv4, reuses act slots
        ob16 = [P.tile([128, OWN, W], BF16, tag=f"act{m}", name=f"ob16{m}")
                for m in range(MT)]
        ofsb = P.tile([128, OWN * W], F32, tag="ofsb", name="ofsb")

        stats1 = P.tile([128, 2 * MT], F32, tag="stats1", name="stats1")
        stats2 = P.tile([128, 2 * MT], F32, tag="stats2", name="stats2")
        sqscr = P.tile([128, OWN, W], F32, tag="sqscr", name="sqscr")

        def dump_bf16(tiles, rows):
            """Debug: copy bf16 tiles (owned rows) to out_sh as f32."""
            for m, t in enumerate(tiles):
                outf = FO.tile([128, OWN, W], F32, tag="outf", name=f"dmp{m}")
                nc.scalar.copy(outf[:], t[:, rows: rows + OWN, :])
                nc.sync.dma_start(out_d.ap()[m], outf[:])

        if KSTAGE == 0:  # DMA in + dump out only, no compute
            dump_bf16([xp[0][:, :AR, :W] for _ in range(MT)], 1)
            return

        if KSTAGE == 5050:  # one psum tile, one weight chunk of matmuls
            pt = PS.tile([128, 8, W], F32, tag="pt", name="pt0")
            wt = WP.tile([128, CHUNK, 128], BF16, tag="wchunk", name="wt0")
            nc.sync.dma_start(wt[:], wa_d.ap()[0, 0])
            for s in range(CHUNK):
                ct, ki, kj = _decode(s)
                nc.tensor.matmul(pt[:], wt[:, s, :],
                                 xp[ct][:, ki: ki + 8, kj: kj + W],
                                 start=(s == 0), stop=(s == CHUNK - 1))
            nc.scalar.copy(csb[0][:, 0:8, :], pt[:])
            dump_bf16([csb[0] for _ in range(MT)], 1)
            return

        # ---- conva: 3x3, 2048 -> 512 over 34 rows ----
        ps_a = {}
        with nc.named_scope("conva"):
            for m in range(MT):
                for n, (r0, nr) in enumerate(NT_A):
                    ps_a[m, n] = PS.tile([128, nr, W], F32, tag="pt",
                                         name=f"psa{m}_{n}")
                for kb in range(4):
                    wt = WP.tile([128, CHUNK, 128], BF16, tag="wchunk",
                                 name=f"wa{m}_{kb}")
                    nc.sync.dma_start(wt[:], wa_d.ap()[m, kb])
                    for s in range(CHUNK):
                        step = kb * CHUNK + s
                        ct, ki, kj = _decode(step)
                        # weight-stationary: all n-tiles share one LDWEIGHTS
                        for n, (r0, nr) in enumerate(NT_A):
                            nc.tensor.matmul(
                                ps_a[m, n][:],
                                wt[:, s, :],
                                xp[ct][:, r0 + ki: r0 + ki + nr, kj: kj + W],
                                start=(step == 0),
                                stop=(step == STEPS_A - 1),
                            )
                # evacuate PSUM -> bf16 SBUF
                for n, (r0, nr) in enumerate(NT_A):
                    nc.scalar.copy(csb[m][:, r0: r0 + nr, :], ps_a[m, n][:])
                # partial BN stats over owned rows (1..33)
                own = csb[m][:, 1: 1 + OWN, :]
                if KSTAGE != 51:
                    nc.vector.tensor_reduce(
                        stats1[:, m: m + 1], own, axis=mybir.AxisListType.XY,
                        op=ALU.add)
                if KSTAGE not in (51, 52):
                    # sumsq: square then reduce (tensor_tensor_reduce faults
                    # the DVE on this runtime)
                    nc.vector.tensor_mul(sqscr[:], own, own)
                    nc.vector.tensor_reduce(
                        stats1[:, MT + m: MT + m + 1], sqscr[:],
                        axis=mybir.AxisListType.XY, op=ALU.add)

        if KSTAGE in (1, 51, 52):
            dump_bf16(csb, 1)
            return

        # ---- BN1: global stats via AllReduce, then scale/shift ----
        sc1, sh1 = _bn_scales(nc, tc, DR, P, stats1, ga, ba, "bn1")

        # ---- BN1 apply: act = relu(conva * sc1 + sh1), bf16 ----
        with nc.named_scope("bn1_apply"):
            for m in range(MT):
                nc.scalar.activation(act[m][:], csb[m][:], AF.Relu,
                                     bias=sh1[:, m: m + 1],
                                     scale=sc1[:, m: m + 1])

        if KSTAGE == 2:
            dump_bf16(act, 1)
            return

        # ---- conv1 m=0, x-part first (overlaps BN1/convb latency) ----
        ps_1 = {}

        def conv1_chunks(m, kbs, last_step=STEPS_1 - 1):
            for n in range(len(NT_1)):
                if (m, n) not in ps_1:
                    ps_1[m, n] = PS.tile([128, 8, W], F32, tag="pt",
                                         name=f"ps1{m}_{n}")
            for kb in kbs:
                wt = WP.tile([128, CHUNK, 128], BF16, tag="wchunk",
                             name=f"w1{m}_{kb}")
                nc.sync.dma_start(wt[:], w1_d.ap()[m, kb])
                for s in range(CHUNK):
                    step = kb * CHUNK + s
                    ct, ki, kj = _decode(step)
                    for n, (r0, nr) in enumerate(NT_1):
                        if ct < CT_X:  # x part of the concat
                            rhs = xp[ct][:, 1 + r0 + ki: 1 + r0 + ki + nr,
                                         kj: kj + W]
                        else:          # out_b part
                            rhs = obp[ct - CT_X][:, r0 + ki: r0 + ki + nr,
                                                 kj: kj + W]
                        nc.tensor.matmul(
                            ps_1[m, n][:], wt[:, s, :], rhs,
                            start=(step == 0), stop=(step == last_step))

        def conv1_finish(m):
            with nc.named_scope("c1fin"):
                for n, (r0, nr) in enumerate(NT_1):
                    nc.scalar.copy(c1sb[m][:, r0: r0 + nr, :], ps_1[m, n][:])
                nc.vector.tensor_reduce(
                    stats2[:, m: m + 1], c1sb[m][:], axis=mybir.AxisListType.XY,
                    op=ALU.add)
                nc.vector.tensor_mul(sqscr[:], c1sb[m][:], c1sb[m][:])
                nc.vector.tensor_reduce(
                    stats2[:, MT + m: MT + m + 1], sqscr[:],
                    axis=mybir.AxisListType.XY, op=ALU.add)

        if KSTAGE == 25:  # conva + bn1 + conv1 x-part of m0 only
            conv1_chunks(0, range(4), last_step=143)
            conv1_finish(0)
            dump_bf16([c1sb[0]] * MT, 0)
            return

        with nc.named_scope("conv1_m0x"):
            conv1_chunks(0, range(4))

        # ---- convb: 1x1 512->512 + bias, masked, into padded buffer ----
        with nc.named_scope("convb"):
            for mo in range(MT):
                # n-tiles in two groups of <=3 to cap live PSUM banks
                # (conv1 m0 holds 4 banks concurrently)
                for ns in (NT_A[:3], NT_A[3:]):
                    base = 0 if ns is NT_A[:3] else 3
                    pbs = [PS.tile([128, nr, W], F32, tag="pt",
                                   name=f"pb{mo}_{base + j}")
                           for j, (r0, nr) in enumerate(ns)]
                    for kt in range(CT_A):
                        for pb, (r0, nr) in zip(pbs, ns):
                            nc.tensor.matmul(
                                pb[:], wbt[:, kt, mo, :],
                                act[kt][:, r0: r0 + nr, :],
                                start=(kt == 0), stop=(kt == CT_A - 1))
                    for pb, (r0, nr) in zip(pbs, ns):
                        tmpb = EV.tile([128, 8, W], BF16, tag="tmpb",
                                       name=f"tb{mo}{r0}")
                        nc.scalar.add(tmpb[:, :nr, :], pb[:],
                                      add=bb[:, mo: mo + 1])
                        nc.vector.tensor_tensor(
                            obp[mo][:, r0: r0 + nr, 1: 1 + W],
                            tmpb[:, :nr, :], mask[:, r0: r0 + nr, :],
                            op=ALU.mult)

        if KSTAGE == 3:
            dump_bf16(obp, 1)
            return

        # ---- conv1 remainder ----
        with nc.named_scope("conv1"):
            conv1_chunks(0, [4])
            conv1_finish(0)
            for m in range(1, MT):
                conv1_chunks(m, range(5))
                conv1_finish(m)

        if KSTAGE == 4:
            dump_bf16(c1sb, 0)
            return

        # ---- BN2 ----
        sc2, sh2 = _bn_scales(nc, tc, DR, P, stats2, g1, b1, "bn2")

        with nc.named_scope("bn2_apply"):
            for m in range(MT):
                outf = FO.tile([128, OWN, W], F32, tag="outf", name=f"outf{m}")
                nc.scalar.activation(outf[:], c1sb[m][:], AF.Relu,
                                     bias=sh2[:, m: m + 1],
                                     scale=sc2[:, m: m + 1])
                nc.vector.tensor_copy(ob16[m][:], outf[:])
                nc.sync.dma_start(out_d.ap()[m], outf[:])

        if KSTAGE == 5:
            return

        # ---- conv4: 1x1 512->19 + bias ----
        with nc.named_scope("conv4"):
            for n, (r0, nr) in enumerate(NT_1):
                p4 = PS.tile([128, nr, W], F32, tag="pt", name=f"p4{n}")
                for kt in range(CT_A):
                    nc.tensor.matmul(
                        p4[:NCLS, :], w4t[:, kt, :], ob16[kt][:, r0: r0 + nr, :],
                        start=(kt == 0), stop=(kt == CT_A - 1))
                nc.scalar.add(ofsb[:NCLS, (r0 * W): (r0 + nr) * W],
                              p4[:NCLS, :], add=b4[:NCLS, :])
            nc.sync.dma_start(ofin_d.ap(), ofsb[:NCLS, :])


def _bn_scales(nc, tc, DR, P, stats, gamma, beta, name):
    """AllReduce partial stats, return (scale, shift) [128, MT] f32 tiles."""
    ccin = DR.tile([128, 2 * MT], F32, tag=f"{name}in", name=f"{name}in")
    ccout = DR.tile([128, 2 * MT], F32, tag=f"{name}out", name=f"{name}out")
    with nc.named_scope(f"{name}_cc"):
        nc.sync.dma_start(ccin[:], stats[:])
        nc.gpsimd.collective_compute(
            "AllReduce", ALU.add,
            replica_groups=[list(range(NCORES))],
            ins=[ccin.opt()], outs=[ccout.opt()])
        gs = P.tile([128, 2 * MT], F32, tag=f"{name}gs", name=f"{name}gs")
        nc.sync.dma_start(gs[:], ccout[:])

    zcol = P.tile([128, 1], F32, tag=f"{name}z", name=f"{name}z")
    nc.vector.memset(zcol[:], 0)
    ms = P.tile([128, 2 * MT], F32, tag=f"{name}ms", name=f"{name}ms")
    var = P.tile([128, MT], F32, tag=f"{name}var", name=f"{name}var")
    std = P.tile([128, MT], F32, tag=f"{name}std", name=f"{name}std")
    rstd = P.tile([128, MT], F32, tag=f"{name}rstd", name=f"{name}rstd")
    sc = P.tile([128, MT], F32, tag=f"{name}sc", name=f"{name}sc")
    sh = P.tile([128, MT], F32, tag=f"{name}sh", name=f"{name}sh")
    with nc.named_scope(f"{name}_scales"):
        nc.vector.tensor_scalar_mul(ms[:], gs[:], 1.0 / COUNT)
        nc.vector.tensor_mul(var[:], ms[:, 0:MT], ms[:, 0:MT])      # mean^2
        nc.vector.tensor_sub(var[:], ms[:, MT:2 * MT], var[:])      # E[x^2]-m^2
        nc.vector.tensor_scalar_add(var[:], var[:], EPS)
        nc.scalar.activation(std[:], var[:], AF.Sqrt, bias=zcol[:, 0:1])
        nc.vector.reciprocal(rstd[:], std[:])
        nc.vector.tensor_mul(sc[:], gamma[:], rstd[:])              # g * rstd
        nc.vector.tensor_mul(sh[:], ms[:, 0:MT], sc[:])             # m * sc
        nc.vector.tensor_sub(sh[:], beta[:], sh[:])                 # b - m*sc
    return sc, sh


# ----------------------------------------------------------------------------
# Host side: prep per-core inputs, run, assemble outputs
# ----------------------------------------------------------------------------

_NC = None


def _get_nc():
    global _NC
    if _NC is None:
        _NC = _build()
    return _NC


def _bf16(a):
    return np.asarray(a, dtype=np.float32).astype(ml_dtypes.bfloat16)


def _prep(x, Wa, ga, ba, Wb, bb, W1, g1, b1, W4, b4):
    x = np.asarray(x, np.float32)
    # weight transforms (shared across cores)
    wa_t = np.ascontiguousarray(
        _bf16(Wa).reshape(MT, 128, CT_X, 128, 3, 3)
        .transpose(0, 2, 4, 5, 3, 1)                # [m, ct, ki, kj, cp, ml]
        .reshape(MT, 4, CHUNK, 128, 128)
        .transpose(0, 1, 3, 2, 4))                  # [m, kb, cp, s, ml]
    w1_t = np.ascontiguousarray(
        _bf16(W1).reshape(MT, 128, CT_X + CT_A, 128, 3, 3)
        .transpose(0, 2, 4, 5, 3, 1)
        .reshape(MT, 5, CHUNK, 128, 128)
        .transpose(0, 1, 3, 2, 4))
    wb_t = np.ascontiguousarray(
        _bf16(Wb[:, :, 0, 0]).reshape(MT, 128, CT_A, 128).transpose(3, 2, 0, 1))
    w4_t = np.ascontiguousarray(
        _bf16(W4[:, :, 0, 0]).reshape(NCLS, CT_A, 128).transpose(2, 1, 0))

    def cols(v):
        return np.ascontiguousarray(
            np.asarray(v, np.float32).reshape(MT, 128).T)

    b4_c = np.zeros((128, 1), np.float32)
    b4_c[:NCLS, 0] = np.asarray(b4, np.float32)

    xbf = _bf16(x).reshape(B, CT_X, 128, H, W)

    in_maps = []
    for c in range(NCORES):
        b, half = divmod(c, 2)
        r0 = OWN * half
        lo = r0 - 2
        xp = np.zeros((CT_X, 128, XR, XC), ml_dtypes.bfloat16)
        glo, ghi = max(lo, 0), min(lo + XR, H)
        xp[:, :, glo - lo: ghi - lo, 1: 1 + W] = xbf[b, :, :, glo:ghi, :]
        msk = np.ones((128, AR, W), ml_dtypes.bfloat16)
        if half == 0:
            msk[:, 0, :] = 0      # conv-a row 0 is global row -1
        else:
            msk[:, AR - 1, :] = 0  # conv-a row 33 is global row 64
        in_maps.append({
            "xp": xp, "mask": msk, "wa": wa_t, "w1": w1_t, "wb": wb_t,
            "w4": w4_t, "ga": cols(ga), "ba": cols(ba), "bb": cols(bb),
            "g1": cols(g1), "b1": cols(b1), "b4": b4_c,
        })
    return in_maps


def run(inputs, trace=False):
    nc = _get_nc()
    in_maps = _prep(**inputs)
    res = run_bass_kernel_spmd(
        nc, in_maps, core_ids=list(range(NCORES)), trace=trace)
    out = np.empty((B, CMID, H, W), np.float32)
    out_final = np.empty((B, NCLS, H, W), np.float32)
    for c in range(NCORES):
        b, half = divmod(c, 2)
        r0 = OWN * half
        o = res.results[c]["out_sh"].reshape(CMID, OWN, W)
        out[b, :, r0: r0 + OWN, :] = o
        of = res.results[c]["ofin_sh"].reshape(NCLS, OWN, W)
        out_final[b, :, r0: r0 + OWN, :] = of
    return (out, out_final), res


def kernel(**inputs):
    (out, out_final), _ = run(inputs, trace=False)
    return out, out_final


# revision 32
# speedup vs baseline: 1.0393x; 1.0393x over previous
"""Trainium2 Bass kernel for the AttentionBranch conv-BN-ReLU pipeline.

Pipeline (reference):
    conva: 3x3 conv (2048->512, pad 1, no bias) -> sync-BN -> ReLU
    convb: 1x1 conv (512->512, bias)
    concat([x, convb_out]) -> conv1: 3x3 conv (2560->512, pad 1, no bias)
        -> sync-BN -> ReLU  (= out)
    conv4: 1x1 conv (512->19, bias)  (= out_final)

Sharding: data-parallel over (batch=4) x (H halves=2) = 8 shards, one per
NeuronCore. Each core computes 34 conv-a rows (its 32 owned rows + 1 halo row
each side; rows outside the image are masked) so that conv1's 3x3 stencil
never needs activations from another core. BatchNorm statistics are made
global with two tiny (128x8 f32) AllReduces.

Convs are computed on the PE as accumulations over (c_tile, ki, kj) shifted
matmuls against a zero-padded input resident in SBUF. All matmul inputs are
bf16, accumulation in fp32 PSUM.
"""

import os

import numpy as np
import ml_dtypes

from concourse import bacc, tile, mybir
import concourse.bass as bass
from concourse.bass_utils import run_bass_kernel_spmd

BF16 = mybir.dt.bfloat16
F32 = mybir.dt.float32
AF = mybir.ActivationFunctionType
ALU = mybir.AluOpType

# Problem shape (hardcoded; kernel.py must be self-contained)
B, CIN, CMID, NCLS, H, W = 4, 2048, 512, 19, 64, 64
NCORES = 8
EPS = 1e-5
COUNT = B * H * W  # BN reduction count (global)

# Per-core spatial shard: 32 owned rows, 34 computed conv-a rows,
# padded x buffer of 36 rows x 66 cols.
OWN = 32          # owned output rows per core
AR = 34           # conv-a rows computed (owned + 1 halo each side)
XR, XC = 36, 66   # padded x shard rows/cols
CT_X = CIN // 128         # 16 c-tiles of x
CT_A = CMID // 128        # 4 c-tiles of mid activations
MT = CMID // 128          # 4 m-tiles of conv outputs
STEPS_A = CT_X * 9        # 144 accumulation steps for conva
STEPS_1 = (CT_X + CT_A) * 9  # 180 steps for conv1
CHUNK = 36                # weight-chunk steps per DMA
# conva n-tiles over 34 rows: 4 full (8 rows) + 1 partial (2 rows)
NT_A = [(0, 8), (8, 8), (16, 8), (24, 8), (32, 2)]
# conv1/conv4 n-tiles over 32 rows
NT_1 = [(0, 8), (8, 8), (16, 8), (24, 8)]


def _decode(step):
    return step // 9, (step % 9) // 3, step % 3  # (c_tile, ki, kj)


# Debug bisect: 1=conva, 2=+bn1, 3=+convb, 4=+conv1, 5=+bn2, 99=full
KSTAGE = int(os.environ.get("KSTAGE", "99"))


def _build():
    nc = bacc.Bacc(
        "TRN2",
        target_bir_lowering=False,
        debug=False,
        enable_asserts=False,
        num_devices=NCORES,
    )

    # ---- I/O tensors (per-core) ----
    xp_d = nc.dram_tensor("xp", [CT_X, 128, XR, XC], BF16, kind="ExternalInput")
    mask_d = nc.dram_tensor("mask", [128, AR, W], BF16, kind="ExternalInput")
    wa_d = nc.dram_tensor("wa", [MT, 4, 128, CHUNK, 128], BF16, kind="ExternalInput")
    w1_d = nc.dram_tensor("w1", [MT, 5, 128, CHUNK, 128], BF16, kind="ExternalInput")
    wb_d = nc.dram_tensor("wb", [128, CT_A, MT, 128], BF16, kind="ExternalInput")
    w4_d = nc.dram_tensor("w4", [128, CT_A, NCLS], BF16, kind="ExternalInput")
    ga_d = nc.dram_tensor("ga", [128, MT], F32, kind="ExternalInput")
    ba_d = nc.dram_tensor("ba", [128, MT], F32, kind="ExternalInput")
    bb_d = nc.dram_tensor("bb", [128, MT], F32, kind="ExternalInput")
    g1_d = nc.dram_tensor("g1", [128, MT], F32, kind="ExternalInput")
    b1_d = nc.dram_tensor("b1", [128, MT], F32, kind="ExternalInput")
    b4_d = nc.dram_tensor("b4", [128, 1], F32, kind="ExternalInput")

    out_d = nc.dram_tensor("out_sh", [MT, 128, OWN, W], F32, kind="ExternalOutput")
    ofin_d = nc.dram_tensor("ofin_sh", [NCLS, OWN, W], F32, kind="ExternalOutput")

    with tile.TileContext(nc) as tc:
        _emit(nc, tc, xp_d, mask_d, wa_d, w1_d, wb_d, w4_d,
              ga_d, ba_d, bb_d, g1_d, b1_d, b4_d, out_d, ofin_d)

    nc.compile()
    return nc


def _emit(nc, tc, xp_d, mask_d, wa_d, w1_d, wb_d, w4_d,
          ga_d, ba_d, bb_d, g1_d, b1_d, b4_d, out_d, ofin_d):
    with tc.tile_pool(name="persist", bufs=1) as P, \
         tc.tile_pool(name="wpool", bufs=3) as WP, \
         tc.tile_pool(name="ev", bufs=2) as EV, \
         tc.tile_pool(name="fout", bufs=2) as FO, \
         tc.tile_pool(name="psum", bufs=8, space="PSUM") as PS, \
         tc.tile_pool(name="dram", bufs=1, space="DRAM") as DR:

        # weight-chunk streaming (shared slots for conva/conv1 chunks)
        chunk_tiles = {}

        def load_chunk(which, m, kb):
            key = (which, m, kb)
            if key not in chunk_tiles:
                wt = WP.tile([128, CHUNK, 128], BF16, tag="wchunk",
                             name=f"w{which}{m}_{kb}")
                src = wa_d if which == "a" else w1_d
                nc.sync.dma_start(wt[:], src.ap()[m, kb])
                chunk_tiles[key] = wt
            return chunk_tiles[key]

        # ---- persistent SBUF tensors + input DMAs ----
        # first x c-tile and first weight chunks lead so conva starts ASAP
        xp = []
        for ct in range(CT_X):
            t = P.tile([128, XR, XC], BF16, tag=f"xp{ct}", name=f"xp{ct}")
            xp.append(t)
        nc.sync.dma_start(xp[0][:], xp_d.ap()[0])
        load_chunk("a", 0, 0)
        nc.sync.dma_start(xp[1][:], xp_d.ap()[1])
        load_chunk("a", 0, 1)
        for ct in range(2, CT_X):
            nc.sync.dma_start(xp[ct][:], xp_d.ap()[ct])
        mask = P.tile([128, AR, W], BF16, tag="mask", name="mask")
        nc.sync.dma_start(mask[:], mask_d.ap())
        wbt = P.tile([128, CT_A, MT, 128], BF16, tag="wbt", name="wbt")
        nc.sync.dma_start(wbt[:], wb_d.ap())
        w4t = P.tile([128, CT_A, NCLS], BF16, tag="w4t", name="w4t")
        nc.sync.dma_start(w4t[:], w4_d.ap())

        def small(name, src=None, cols=MT):
            t = P.tile([128, cols], F32, tag=name, name=name)
            if src is not None:
                nc.sync.dma_start(t[:], src.ap())
            return t

        ga = small("ga", ga_d)
        ba = small("ba", ba_d)
        bb = small("bb", bb_d)
        g1 = small("g1", g1_d)
        b1 = small("b1", b1_d)
        b4 = small("b4", b4_d, cols=1)

        # out_b padded buffers (zero borders), conv-a sbuf, activations
        obp = []
        for m in range(MT):
            t = P.tile([128, AR, XC], BF16, tag=f"obp{m}", name=f"obp{m}")
            nc.vector.memset(t[:], 0)
            obp.append(t)
        csb = [P.tile([128, AR, W], BF16, tag=f"csb{m}", name=f"csb{m}")
               for m in range(MT)]
        act = [P.tile([128, AR, W], BF16, tag=f"act{m}", name=f"act{m}")
               for m in range(MT)]
        # conv1 output (bf16) reuses the conv-a slots (disjoint lifetime)
        c1sb = [P.tile([128, OWN, W], BF16, tag=f"csb{m}", name=f"c1sb{m}")
                for m in range(MT)]
        # bf16 copy of `out` for conv4, reuses act slots
        ob16 = [P.tile([128, OWN, W], BF16, tag=f"act{m}", name=f"ob16{m}")
                for m in range(MT)]
        ofsb = P.tile([128, OWN * W], F32, tag="ofsb", name="ofsb")

        stats1 = P.tile([128, 2 * MT], F32, tag="stats1", name="stats1")
        stats2 = P.tile([128, 2 * MT], F32, tag="stats2", name="stats2")
        sqscr = P.tile([128, OWN, W], BF16, tag="sqscr", name="sqscr")
        z0 = P.tile([128, 1], F32, tag="z0", name="z0")
        nc.vector.memset(z0[:], 0)

        def dump_bf16(tiles, rows):
            """Debug: copy bf16 tiles (owned rows) to out_sh as f32."""
            for m, t in enumerate(tiles):
                outf = FO.tile([128, OWN, W], F32, tag="outf", name=f"dmp{m}")
                nc.scalar.copy(outf[:], t[:, rows: rows + OWN, :])
                nc.sync.dma_start(out_d.ap()[m], outf[:])

        if KSTAGE == 0:  # DMA in + dump out only, no compute
            dump_bf16([xp[0][:, :AR, :W] for _ in range(MT)], 1)
            return

        if KSTAGE == 5050:  # one psum tile, one weight chunk of matmuls
            pt = PS.tile([128, 8, W], F32, tag="pt", name="pt0")
            wt = WP.tile([128, CHUNK, 128], BF16, tag="wchunk", name="wt0")
            nc.sync.dma_start(wt[:], wa_d.ap()[0, 0])
            for s in range(CHUNK):
                ct, ki, kj = _decode(s)
                nc.tensor.matmul(pt[:], wt[:, s, :],
                                 xp[ct][:, ki: ki + 8, kj: kj + W],
                                 start=(s == 0), stop=(s == CHUNK - 1))
            nc.scalar.copy(csb[0][:, 0:8, :], pt[:])
            dump_bf16([csb[0] for _ in range(MT)], 1)
            return

        # ---- conva: 3x3, 2048 -> 512 over 34 rows ----
        ps_a = {}
        with nc.named_scope("conva"):
            for m in range(MT):
                for n, (r0, nr) in enumerate(NT_A):
                    ps_a[m, n] = PS.tile([128, nr, W], F32, tag="pt",
                                         name=f"psa{m}_{n}")
                for kb in range(4):
                    wt = load_chunk("a", m, kb)
                    for s in range(CHUNK):
                        step = kb * CHUNK + s
                        ct, ki, kj = _decode(step)
                        # weight-stationary: all n-tiles share one LDWEIGHTS
                        for n, (r0, nr) in enumerate(NT_A):
                            nc.tensor.matmul(
                                ps_a[m, n][:],
                                wt[:, s, :],
                                xp[ct][:, r0 + ki: r0 + ki + nr, kj: kj + W],
                                start=(step == 0),
                                stop=(step == STEPS_A - 1),
                            )
                # evacuate PSUM -> bf16 SBUF
                for n, (r0, nr) in enumerate(NT_A):
                    nc.scalar.copy(csb[m][:, r0: r0 + nr, :], ps_a[m, n][:])
                # partial BN stats over owned rows (1..33)
                own = csb[m][:, 1: 1 + OWN, :]
                if KSTAGE != 51:
                    nc.vector.tensor_reduce(
                        stats1[:, m: m + 1], own, axis=mybir.AxisListType.XY,
                        op=ALU.add)
                if KSTAGE not in (51, 52):
                    # sumsq on ACT: square with free-dim accumulate
                    # (tensor_tensor_reduce faults the DVE on this runtime)
                    nc.scalar.activation(
                        sqscr[:], own, AF.Square, bias=z0[:, 0:1],
                        accum_out=stats1[:, MT + m: MT + m + 1])

        if KSTAGE in (1, 51, 52):
            dump_bf16(csb, 1)
            return

        # ---- BN1: global stats via AllReduce, then scale/shift ----
        sc1, sh1 = _bn_scales(nc, tc, DR, P, stats1, ga, ba, "bn1")

        # ---- BN1 apply: act = relu(conva * sc1 + sh1), bf16 ----
        with nc.named_scope("bn1_apply"):
            for m in range(MT):
                nc.scalar.activation(act[m][:], csb[m][:], AF.Relu,
                                     bias=sh1[:, m: m + 1],
                                     scale=sc1[:, m: m + 1])

        if KSTAGE == 2:
            dump_bf16(act, 1)
            return

        # ---- conv1 m=0, x-part first (overlaps BN1/convb latency) ----
        ps_1 = {}

        def conv1_chunks(m, kbs, last_step=STEPS_1 - 1):
            for n in range(len(NT_1)):
                if (m, n) not in ps_1:
                    ps_1[m, n] = PS.tile([128, 8, W], F32, tag="pt",
                                         name=f"ps1{m}_{n}")
            for kb in kbs:
                wt = load_chunk("1", m, kb)
                for s in range(CHUNK):
                    step = kb * CHUNK + s
                    ct, ki, kj = _decode(step)
                    for n, (r0, nr) in enumerate(NT_1):
                        if ct < CT_X:  # x part of the concat
                            rhs = xp[ct][:, 1 + r0 + ki: 1 + r0 + ki + nr,
                                         kj: kj + W]
                        else:          # out_b part
                            rhs = obp[ct - CT_X][:, r0 + ki: r0 + ki + nr,
                                                 kj: kj + W]
                        nc.tensor.matmul(
                            ps_1[m, n][:], wt[:, s, :], rhs,
                            start=(step == 0), stop=(step == last_step))

        def conv1_finish(m):
            with nc.named_scope("c1fin"):
                for n, (r0, nr) in enumerate(NT_1):
                    nc.scalar.copy(c1sb[m][:, r0: r0 + nr, :], ps_1[m, n][:])
                nc.vector.tensor_reduce(
                    stats2[:, m: m + 1], c1sb[m][:], axis=mybir.AxisListType.XY,
                    op=ALU.add)
                nc.scalar.activation(
                    sqscr[:], c1sb[m][:], AF.Square, bias=z0[:, 0:1],
                    accum_out=stats2[:, MT + m: MT + m + 1])

        if KSTAGE == 25:  # conva + bn1 + conv1 x-part of m0 only
            conv1_chunks(0, range(4), last_step=143)
            conv1_finish(0)
            dump_bf16([c1sb[0]] * MT, 0)
            return

        with nc.named_scope("conv1_m0x"):
            conv1_chunks(0, range(4))

        # ---- convb: 1x1 512->512 + bias, masked, into padded buffer ----
        with nc.named_scope("convb"):
            for mo in range(MT):
                # n-tiles in two groups of <=3 to cap live PSUM banks
                # (conv1 m0 holds 4 banks concurrently)
                for ns in (NT_A[:3], NT_A[3:]):
                    base = 0 if ns is NT_A[:3] else 3
                    pbs = [PS.tile([128, nr, W], F32, tag="pt",
                                   name=f"pb{mo}_{base + j}")
                           for j, (r0, nr) in enumerate(ns)]
                    for kt in range(CT_A):
                        for pb, (r0, nr) in zip(pbs, ns):
                            nc.tensor.matmul(
                                pb[:], wbt[:, kt, mo, :],
                                act[kt][:, r0: r0 + nr, :],
                                start=(kt == 0), stop=(kt == CT_A - 1))
                    for pb, (r0, nr) in zip(pbs, ns):
                        tmpb = EV.tile([128, 8, W], BF16, tag="tmpb",
                                       name=f"tb{mo}{r0}")
                        nc.scalar.add(tmpb[:, :nr, :], pb[:],
                                      add=bb[:, mo: mo + 1])
                        nc.vector.tensor_tensor(
                            obp[mo][:, r0: r0 + nr, 1: 1 + W],
                            tmpb[:, :nr, :], mask[:, r0: r0 + nr, :],
                            op=ALU.mult)

        if KSTAGE == 3:
            dump_bf16(obp, 1)
            return

        # ---- conv1 remainder ----
        with nc.named_scope("conv1"):
            conv1_chunks(0, [4])
            conv1_finish(0)
            for m in range(1, MT):
                conv1_chunks(m, range(5))
                conv1_finish(m)

        if KSTAGE == 4:
            dump_bf16(c1sb, 0)
            return

        # ---- BN2 ----
        sc2, sh2 = _bn_scales(nc, tc, DR, P, stats2, g1, b1, "bn2")

        # ---- BN2 apply + conv4 (1x1 512->19) interleaved per m-tile ----
        p4 = {}
        with nc.named_scope("bn2_apply"):
            for m in range(MT):
                outf = FO.tile([128, OWN, W], F32, tag="outf", name=f"outf{m}")
                nc.scalar.activation(outf[:], c1sb[m][:], AF.Relu,
                                     bias=sh2[:, m: m + 1],
                                     scale=sc2[:, m: m + 1])
                nc.vector.tensor_copy(ob16[m][:], outf[:])
                nc.sync.dma_start(out_d.ap()[m], outf[:])
                if KSTAGE == 5:
                    continue
                for n, (r0, nr) in enumerate(NT_1):
                    if n not in p4:
                        p4[n] = PS.tile([128, nr, W], F32, tag="pt",
                                        name=f"p4{n}")
                    nc.tensor.matmul(
                        p4[n][:NCLS, :], w4t[:, m, :],
                        ob16[m][:, r0: r0 + nr, :],
                        start=(m == 0), stop=(m == MT - 1))

        if KSTAGE == 5:
            return

        with nc.named_scope("conv4"):
            for n, (r0, nr) in enumerate(NT_1):
                nc.scalar.add(ofsb[:NCLS, (r0 * W): (r0 + nr) * W],
                              p4[n][:NCLS, :], add=b4[:NCLS, :])
            nc.sync.dma_start(ofin_d.ap(), ofsb[:NCLS, :])


def _bn_scales(nc, tc, DR, P, stats, gamma, beta, name):
    """AllReduce partial stats, return (scale, shift) [128, MT] f32 tiles."""
    ccin = DR.tile([128, 2 * MT], F32, tag=f"{name}in", name=f"{name}in")
    ccout = DR.tile([128, 2 * MT], F32, tag=f"{name}out", name=f"{name}out")
    with nc.named_scope(f"{name}_cc"):
        nc.sync.dma_start(ccin[:], stats[:])
        nc.gpsimd.collective_compute(
            "AllReduce", ALU.add,
            replica_groups=[list(range(NCORES))],
            ins=[ccin.opt()], outs=[ccout.opt()])
        gs = P.tile([128, 2 * MT], F32, tag=f"{name}gs", name=f"{name}gs")
        nc.sync.dma_start(gs[:], ccout[:])

    zcol = P.tile([128, 1], F32, tag=f"{name}z", name=f"{name}z")
    nc.vector.memset(zcol[:], 0)
    ms = P.tile([128, 2 * MT], F32, tag=f"{name}ms", name=f"{name}ms")
    var = P.tile([128, MT], F32, tag=f"{name}var", name=f"{name}var")
    std = P.tile([128, MT], F32, tag=f"{name}std", name=f"{name}std")
    rstd = P.tile([128, MT], F32, tag=f"{name}rstd", name=f"{name}rstd")
    sc = P.tile([128, MT], F32, tag=f"{name}sc", name=f"{name}sc")
    sh = P.tile([128, MT], F32, tag=f"{name}sh", name=f"{name}sh")
    with nc.named_scope(f"{name}_scales"):
        nc.vector.tensor_scalar_mul(ms[:], gs[:], 1.0 / COUNT)
        nc.vector.tensor_mul(var[:], ms[:, 0:MT], ms[:, 0:MT])      # mean^2
        nc.vector.tensor_sub(var[:], ms[:, MT:2 * MT], var[:])      # E[x^2]-m^2
        nc.vector.tensor_scalar_add(var[:], var[:], EPS)
        nc.scalar.activation(std[:], var[:], AF.Sqrt, bias=zcol[:, 0:1])
        nc.vector.reciprocal(rstd[:], std[:])
        nc.vector.tensor_mul(sc[:], gamma[:], rstd[:])              # g * rstd
        nc.vector.tensor_mul(sh[:], ms[:, 0:MT], sc[:])             # m * sc
        nc.vector.tensor_sub(sh[:], beta[:], sh[:])                 # b - m*sc
    return sc, sh


# ----------------------------------------------------------------------------
# Host side: prep per-core inputs, run, assemble outputs
# ----------------------------------------------------------------------------

_NC = None


def _get_nc():
    global _NC
    if _NC is None:
        _NC = _build()
    return _NC


def _bf16(a):
    return np.asarray(a, dtype=np.float32).astype(ml_dtypes.bfloat16)


def _prep(x, Wa, ga, ba, Wb, bb, W1, g1, b1, W4, b4):
    x = np.asarray(x, np.float32)
    # weight transforms (shared across cores)
    wa_t = np.ascontiguousarray(
        _bf16(Wa).reshape(MT, 128, CT_X, 128, 3, 3)
        .transpose(0, 2, 4, 5, 3, 1)                # [m, ct, ki, kj, cp, ml]
        .reshape(MT, 4, CHUNK, 128, 128)
        .transpose(0, 1, 3, 2, 4))                  # [m, kb, cp, s, ml]
    w1_t = np.ascontiguousarray(
        _bf16(W1).reshape(MT, 128, CT_X + CT_A, 128, 3, 3)
        .transpose(0, 2, 4, 5, 3, 1)
        .reshape(MT, 5, CHUNK, 128, 128)
        .transpose(0, 1, 3, 2, 4))
    wb_t = np.ascontiguousarray(
        _bf16(Wb[:, :, 0, 0]).reshape(MT, 128, CT_A, 128).transpose(3, 2, 0, 1))
    w4_t = np.ascontiguousarray(
        _bf16(W4[:, :, 0, 0]).reshape(NCLS, CT_A, 128).transpose(2, 1, 0))

    def cols(v):
        return np.ascontiguousarray(
            np.asarray(v, np.float32).reshape(MT, 128).T)

    b4_c = np.zeros((128, 1), np.float32)
    b4_c[:NCLS, 0] = np.asarray(b4, np.float32)

    xbf = _bf16(x).reshape(B, CT_X, 128, H, W)

    in_maps = []
    for c in range(NCORES):
        b, half = divmod(c, 2)
        r0 = OWN * half
        lo = r0 - 2
        xp = np.zeros((CT_X, 128, XR, XC), ml_dtypes.bfloat16)
        glo, ghi = max(lo, 0), min(lo + XR, H)
        xp[:, :, glo - lo: ghi - lo, 1: 1 + W] = xbf[b, :, :, glo:ghi, :]
        msk = np.ones((128, AR, W), ml_dtypes.bfloat16)
        if half == 0:
            msk[:, 0, :] = 0      # conv-a row 0 is global row -1
        else:
            msk[:, AR - 1, :] = 0  # conv-a row 33 is global row 64
        in_maps.append({
            "xp": xp, "mask": msk, "wa": wa_t, "w1": w1_t, "wb": wb_t,
            "w4": w4_t, "ga": cols(ga), "ba": cols(ba), "bb": cols(bb),
            "g1": cols(g1), "b1": cols(b1), "b4": b4_c,
        })
    return in_maps


def run(inputs, trace=False):
    nc = _get_nc()
    in_maps = _prep(**inputs)
    res = run_bass_kernel_spmd(
        nc, in_maps, core_ids=list(range(NCORES)), trace=trace)
    out = np.empty((B, CMID, H, W), np.float32)
    out_final = np.empty((B, NCLS, H, W), np.float32)
    for c in range(NCORES):
        b, half = divmod(c, 2)
        r0 = OWN * half
        o = res.results[c]["out_sh"].reshape(CMID, OWN, W)
        out[b, :, r0: r0 + OWN, :] = o
        of = res.results[c]["ofin_sh"].reshape(NCLS, OWN, W)
        out_final[b, :, r0: r0 + OWN, :] = of
    return (out, out_final), res


def kernel(**inputs):
    (out, out_final), _ = run(inputs, trace=False)
    return out, out_final


# revision 34
# speedup vs baseline: 1.0515x; 1.0117x over previous
"""Trainium2 Bass kernel for the AttentionBranch conv-BN-ReLU pipeline.

Pipeline (reference):
    conva: 3x3 conv (2048->512, pad 1, no bias) -> sync-BN -> ReLU
    convb: 1x1 conv (512->512, bias)
    concat([x, convb_out]) -> conv1: 3x3 conv (2560->512, pad 1, no bias)
        -> sync-BN -> ReLU  (= out)
    conv4: 1x1 conv (512->19, bias)  (= out_final)

Sharding: data-parallel over (batch=4) x (H halves=2) = 8 shards, one per
NeuronCore; each core computes exactly its 32 owned rows. BatchNorm stats are
made global with two tiny (128x8 f32) AllReduces. conv1's 3x3 stencil needs
one out_b halo row from the paired core; the pairs exchange both boundary
rows with a small AllGather (hidden under conv1's x-channel matmuls) and a
per-core 0/1 mask zeroes the row that is global padding.

Convs run on the PE as accumulations over (c_tile, ki, kj) shifted matmuls
against zero-padded SBUF-resident inputs. All matmul inputs are bf16,
accumulation in fp32 PSUM.
"""

import numpy as np
import ml_dtypes

from concourse import bacc, tile, mybir
import concourse.bass as bass
from concourse.bass_utils import run_bass_kernel_spmd

BF16 = mybir.dt.bfloat16
F32 = mybir.dt.float32
AF = mybir.ActivationFunctionType
ALU = mybir.AluOpType

# Problem shape (hardcoded; kernel.py must be self-contained)
B, CIN, CMID, NCLS, H, W = 4, 2048, 512, 19, 64, 64
NCORES = 8
EPS = 1e-5
COUNT = B * H * W  # BN reduction count (global)

OWN = 32          # owned output rows per core
XR, XC = 34, 66   # padded x shard rows/cols (rows r0-1 .. r0+32, cols -1..64)
OBR = 34          # out_b padded buffer rows (own 32 + 1 halo each side)
CT_X = CIN // 128         # 16 c-tiles of x
CT_A = CMID // 128        # 4 c-tiles of mid activations
MT = CMID // 128          # 4 m-tiles of conv outputs
STEPS_A = CT_X * 9        # 144 accumulation steps for conva
STEPS_1 = (CT_X + CT_A) * 9  # 180 steps for conv1
CHUNK = 36                # weight-chunk steps per kb block
NT = [(0, 8), (8, 8), (16, 8), (24, 8)]  # n-tiles over the 32 owned rows


def _decode(step):
    return step // 9, (step % 9) // 3, step % 3  # (c_tile, ki, kj)


def _chunks_for(m):
    """(kb, s0, ns) weight-chunk specs; m=0 leads with a small chunk so the
    first matmul can start after a ~400KB DMA."""
    if m == 0:
        return [(0, 0, 12), (0, 12, 24), (1, 0, 36), (2, 0, 36), (3, 0, 36)]
    return [(kb, 0, 36) for kb in range(4)]


def _build():
    nc = bacc.Bacc(
        "TRN2",
        target_bir_lowering=False,
        debug=False,
        enable_asserts=False,
        num_devices=NCORES,
    )

    # ---- I/O tensors (per-core) ----
    xp_d = nc.dram_tensor("xp", [CT_X, 128, XR, XC], BF16, kind="ExternalInput")
    mask_d = nc.dram_tensor("mask", [128, OBR, W], BF16, kind="ExternalInput")
    wa_d = nc.dram_tensor("wa", [MT, 4, 128, CHUNK, 128], BF16, kind="ExternalInput")
    w1_d = nc.dram_tensor("w1", [MT, 5, 128, CHUNK, 128], BF16, kind="ExternalInput")
    wb_d = nc.dram_tensor("wb", [128, CT_A, MT, 128], BF16, kind="ExternalInput")
    w4_d = nc.dram_tensor("w4", [128, CT_A, NCLS], BF16, kind="ExternalInput")
    ga_d = nc.dram_tensor("ga", [128, MT], F32, kind="ExternalInput")
    ba_d = nc.dram_tensor("ba", [128, MT], F32, kind="ExternalInput")
    bb_d = nc.dram_tensor("bb", [128, MT], F32, kind="ExternalInput")
    g1_d = nc.dram_tensor("g1", [128, MT], F32, kind="ExternalInput")
    b1_d = nc.dram_tensor("b1", [128, MT], F32, kind="ExternalInput")
    b4_d = nc.dram_tensor("b4", [128, 1], F32, kind="ExternalInput")

    out_d = nc.dram_tensor("out_sh", [MT, 128, OWN, W], F32, kind="ExternalOutput")
    ofin_d = nc.dram_tensor("ofin_sh", [NCLS, OWN, W], F32, kind="ExternalOutput")

    with tile.TileContext(nc) as tc:
        _emit(nc, tc, xp_d, mask_d, wa_d, w1_d, wb_d, w4_d,
              ga_d, ba_d, bb_d, g1_d, b1_d, b4_d, out_d, ofin_d)

    nc.compile()
    return nc


def _emit(nc, tc, xp_d, mask_d, wa_d, w1_d, wb_d, w4_d,
          ga_d, ba_d, bb_d, g1_d, b1_d, b4_d, out_d, ofin_d):
    with tc.tile_pool(name="persist", bufs=1) as P, \
         tc.tile_pool(name="wpool", bufs=3) as WP, \
         tc.tile_pool(name="fout", bufs=2) as FO, \
         tc.tile_pool(name="psum", bufs=8, space="PSUM") as PS, \
         tc.tile_pool(name="dram", bufs=1, space="DRAM") as DR:

        # weight-chunk streaming (shared slots for conva/conv1 chunks)
        chunk_tiles = {}

        def load_chunk(which, m, kb, s0, ns):
            key = (which, m, kb, s0)
            if key not in chunk_tiles:
                wt = WP.tile([128, ns, 128], BF16, tag="wchunk",
                             name=f"w{which}{m}_{kb}_{s0}")
                src = wa_d if which == "a" else w1_d
                nc.sync.dma_start(wt[:], src.ap()[m, kb][:, s0: s0 + ns, :])
                chunk_tiles[key] = wt
            return chunk_tiles[key]

        # ---- persistent SBUF tensors + input DMAs ----
        # first weight chunk + first x c-tile lead so conva starts ASAP
        xp = [P.tile([128, XR, XC], BF16, tag=f"xp{ct}", name=f"xp{ct}")
              for ct in range(CT_X)]
        nc.sync.dma_start(xp[0][:], xp_d.ap()[0])
        load_chunk("a", 0, 0, 0, 12)
        nc.sync.dma_start(xp[1][:], xp_d.ap()[1])
        load_chunk("a", 0, 0, 12, 24)
        for ct in range(2, CT_X):
            nc.sync.dma_start(xp[ct][:], xp_d.ap()[ct])
        mask = P.tile([128, OBR, W], BF16, tag="mask", name="mask")
        nc.sync.dma_start(mask[:], mask_d.ap())
        wbt = P.tile([128, CT_A, MT, 128], BF16, tag="wbt", name="wbt")
        nc.sync.dma_start(wbt[:], wb_d.ap())
        w4t = P.tile([128, CT_A, NCLS], BF16, tag="w4t", name="w4t")
        nc.sync.dma_start(w4t[:], w4_d.ap())

        def small(name, src=None, cols=MT):
            t = P.tile([128, cols], F32, tag=name, name=name)
            if src is not None:
                nc.sync.dma_start(t[:], src.ap())
            return t

        ga = small("ga", ga_d)
        ba = small("ba", ba_d)
        bb = small("bb", bb_d)
        g1 = small("g1", g1_d)
        b1 = small("b1", b1_d)
        b4 = small("b4", b4_d, cols=1)

        # out_b padded buffers (zero borders), conv-a sbuf, activations
        obp = []
        for m in range(MT):
            t = P.tile([128, OBR, XC], BF16, tag=f"obp{m}", name=f"obp{m}")
            nc.vector.memset(t[:], 0)
            obp.append(t)
        csb = [P.tile([128, OWN, W], BF16, tag=f"csb{m}", name=f"csb{m}")
               for m in range(MT)]
        act = [P.tile([128, OWN, W], BF16, tag=f"act{m}", name=f"act{m}")
               for m in range(MT)]
        # conv1 output (bf16) reuses the conv-a slots (disjoint lifetime)
        c1sb = [P.tile([128, OWN, W], BF16, tag=f"csb{m}", name=f"c1sb{m}")
                for m in range(MT)]
        # bf16 copy of `out` for conv4, reuses act slots
        ob16 = [P.tile([128, OWN, W], BF16, tag=f"act{m}", name=f"ob16{m}")
                for m in range(MT)]
        ofsb = P.tile([128, OWN * W], F32, tag="ofsb", name="ofsb")

        stats1 = P.tile([128, 2 * MT], F32, tag="stats1", name="stats1")
        stats2 = P.tile([128, 2 * MT], F32, tag="stats2", name="stats2")
        sqscr = P.tile([128, OWN, W], BF16, tag="sqscr", name="sqscr")
        z0 = P.tile([128, 1], F32, tag="z0", name="z0")
        nc.vector.memset(z0[:], 0)

        # ---- conva: 3x3, 2048 -> 512 over the 32 owned rows ----
        ps_a = {}
        with nc.named_scope("conva"):
            for m in range(MT):
                for n, (r0, nr) in enumerate(NT):
                    ps_a[m, n] = PS.tile([128, nr, W], F32, tag="pt",
                                         name=f"psa{m}_{n}")
                for kb, s0, ns in _chunks_for(m):
                    wt = load_chunk("a", m, kb, s0, ns)
                    for s in range(ns):
                        step = kb * CHUNK + s0 + s
                        ct, ki, kj = _decode(step)
                        for n, (r0, nr) in enumerate(NT):
                            nc.tensor.matmul(
                                ps_a[m, n][:],
                                wt[:, s, :],
                                xp[ct][:, r0 + ki: r0 + ki + nr, kj: kj + W],
                                start=(step == 0),
                                stop=(step == STEPS_A - 1),
                            )
                # evacuate PSUM -> bf16 SBUF + partial BN stats
                for n, (r0, nr) in enumerate(NT):
                    nc.scalar.copy(csb[m][:, r0: r0 + nr, :], ps_a[m, n][:])
                nc.vector.tensor_reduce(
                    stats1[:, m: m + 1], csb[m][:], axis=mybir.AxisListType.XY,
                    op=ALU.add)
                # sumsq on ACT: square with free-dim accumulate
                # (tensor_tensor_reduce faults the DVE on this runtime)
                nc.scalar.activation(
                    sqscr[:], csb[m][:], AF.Square, bias=z0[:, 0:1],
                    accum_out=stats1[:, MT + m: MT + m + 1])

        # ---- BN1: global stats via AllReduce, then scale/shift ----
        sc1, sh1 = _bn_scales(nc, tc, DR, P, stats1, ga, ba, "bn1")

        # ---- BN1 apply: act = relu(conva * sc1 + sh1), bf16 ----
        with nc.named_scope("bn1_apply"):
            for m in range(MT):
                nc.scalar.activation(act[m][:], csb[m][:], AF.Relu,
                                     bias=sh1[:, m: m + 1],
                                     scale=sc1[:, m: m + 1])

        # ---- conv1 m=0, x-part first (overlaps BN1/convb/exchange) ----
        ps_1 = {}

        def conv1_chunks(m, kbs):
            for n in range(len(NT)):
                if (m, n) not in ps_1:
                    ps_1[m, n] = PS.tile([128, 8, W], F32, tag="pt",
                                         name=f"ps1{m}_{n}")
            for kb in kbs:
                wt = load_chunk("1", m, kb, 0, CHUNK)
                for s in range(CHUNK):
                    step = kb * CHUNK + s
                    ct, ki, kj = _decode(step)
                    for n, (r0, nr) in enumerate(NT):
                        if ct < CT_X:  # x part of the concat
                            rhs = xp[ct][:, r0 + ki: r0 + ki + nr, kj: kj + W]
                        else:          # out_b part (padded buffer)
                            rhs = obp[ct - CT_X][:, r0 + ki: r0 + ki + nr,
                                                 kj: kj + W]
                        nc.tensor.matmul(
                            ps_1[m, n][:], wt[:, s, :], rhs,
                            start=(step == 0), stop=(step == STEPS_1 - 1))

        def conv1_finish(m):
            with nc.named_scope("c1fin"):
                for n, (r0, nr) in enumerate(NT):
                    nc.scalar.copy(c1sb[m][:, r0: r0 + nr, :], ps_1[m, n][:])
                nc.vector.tensor_reduce(
                    stats2[:, m: m + 1], c1sb[m][:], axis=mybir.AxisListType.XY,
                    op=ALU.add)
                nc.scalar.activation(
                    sqscr[:], c1sb[m][:], AF.Square, bias=z0[:, 0:1],
                    accum_out=stats2[:, MT + m: MT + m + 1])

        with nc.named_scope("conv1_m0x"):
            conv1_chunks(0, range(4))

        # ---- convb: 1x1 512->512 + bias into obp rows 1..32 ----
        with nc.named_scope("convb"):
            for mo in range(MT):
                for half in (NT[:2], NT[2:]):
                    pbs = [PS.tile([128, nr, W], F32, tag="pt",
                                   name=f"pb{mo}_{r0}")
                           for (r0, nr) in half]
                    for kt in range(CT_A):
                        for pb, (r0, nr) in zip(pbs, half):
                            nc.tensor.matmul(
                                pb[:], wbt[:, kt, mo, :],
                                act[kt][:, r0: r0 + nr, :],
                                start=(kt == 0), stop=(kt == CT_A - 1))
                    for pb, (r0, nr) in zip(pbs, half):
                        nc.scalar.add(
                            obp[mo][:, 1 + r0: 1 + r0 + nr, 1: 1 + W],
                            pb[:], add=bb[:, mo: mo + 1])

        # ---- halo exchange: pairs swap out_b boundary rows (AllGather) ----
        # Every core contributes both boundary rows; every core then places
        # slot[0].row1 -> obp row 0 and slot[1].row0 -> obp row 33. The row
        # that lands on a global-padding position is the core's own row; the
        # mask multiply zeroes it.
        with nc.named_scope("halo_cc"):
            ccxin = DR.tile([128, 2, MT, W], BF16, tag="ccxin", name="ccxin")
            ccxout = DR.tile([2, 128, 2, MT, W], BF16, tag="ccxout",
                             name="ccxout")
            for mo in range(MT):
                nc.sync.dma_start(ccxin[:, 0, mo, :],
                                  obp[mo][:, 1: 2, 1: 1 + W])
                nc.sync.dma_start(ccxin[:, 1, mo, :],
                                  obp[mo][:, OBR - 2: OBR - 1, 1: 1 + W])
            nc.gpsimd.collective_compute(
                "AllGather", ALU.bypass,
                replica_groups=[[2 * p, 2 * p + 1] for p in range(NCORES // 2)],
                ins=[ccxin.opt()], outs=[ccxout.opt()])
            for mo in range(MT):
                nc.sync.dma_start(obp[mo][:, 0: 1, 1: 1 + W],
                                  ccxout[0, :, 1, mo, :])
                nc.sync.dma_start(obp[mo][:, OBR - 1: OBR, 1: 1 + W],
                                  ccxout[1, :, 0, mo, :])
                nc.vector.tensor_tensor(
                    obp[mo][:, 0: 1, 1: 1 + W],
                    obp[mo][:, 0: 1, 1: 1 + W], mask[:, 0: 1, :], op=ALU.mult)
                nc.vector.tensor_tensor(
                    obp[mo][:, OBR - 1: OBR, 1: 1 + W],
                    obp[mo][:, OBR - 1: OBR, 1: 1 + W],
                    mask[:, OBR - 1: OBR, :], op=ALU.mult)

        # ---- conv1 remainder ----
        with nc.named_scope("conv1"):
            conv1_chunks(0, [4])
            conv1_finish(0)
            for m in range(1, MT):
                conv1_chunks(m, range(5))
                conv1_finish(m)

        # ---- BN2 ----
        sc2, sh2 = _bn_scales(nc, tc, DR, P, stats2, g1, b1, "bn2")

        # ---- BN2 apply + conv4 (1x1 512->19) interleaved per m-tile ----
        p4 = {}
        with nc.named_scope("bn2_apply"):
            for m in range(MT):
                outf = FO.tile([128, OWN, W], F32, tag="outf", name=f"outf{m}")
                nc.scalar.activation(outf[:], c1sb[m][:], AF.Relu,
                                     bias=sh2[:, m: m + 1],
                                     scale=sc2[:, m: m + 1])
                nc.vector.tensor_copy(ob16[m][:], outf[:])
                nc.sync.dma_start(out_d.ap()[m], outf[:])
                for n, (r0, nr) in enumerate(NT):
                    if n not in p4:
                        p4[n] = PS.tile([128, nr, W], F32, tag="pt",
                                        name=f"p4{n}")
                    nc.tensor.matmul(
                        p4[n][:NCLS, :], w4t[:, m, :],
                        ob16[m][:, r0: r0 + nr, :],
                        start=(m == 0), stop=(m == MT - 1))

        with nc.named_scope("conv4"):
            for n, (r0, nr) in enumerate(NT):
                nc.scalar.add(ofsb[:NCLS, (r0 * W): (r0 + nr) * W],
                              p4[n][:NCLS, :], add=b4[:NCLS, :])
            nc.sync.dma_start(ofin_d.ap(), ofsb[:NCLS, :])


def _bn_scales(nc, tc, DR, P, stats, gamma, beta, name):
    """AllReduce partial stats, return (scale, shift) [128, MT] f32 tiles."""
    ccin = DR.tile([128, 2 * MT], F32, tag=f"{name}in", name=f"{name}in")
    ccout = DR.tile([128, 2 * MT], F32, tag=f"{name}out", name=f"{name}out")
    with nc.named_scope(f"{name}_cc"):
        nc.sync.dma_start(ccin[:], stats[:])
        nc.gpsimd.collective_compute(
            "AllReduce", ALU.add,
            replica_groups=[list(range(NCORES))],
            ins=[ccin.opt()], outs=[ccout.opt()])
        gs = P.tile([128, 2 * MT], F32, tag=f"{name}gs", name=f"{name}gs")
        nc.sync.dma_start(gs[:], ccout[:])

    epsc = P.tile([128, 1], F32, tag=f"{name}e", name=f"{name}e")
    nc.vector.memset(epsc[:], EPS)
    ms = P.tile([128, 2 * MT], F32, tag=f"{name}ms", name=f"{name}ms")
    var = P.tile([128, MT], F32, tag=f"{name}var", name=f"{name}var")
    std = P.tile([128, MT], F32, tag=f"{name}std", name=f"{name}std")
    rstd = P.tile([128, MT], F32, tag=f"{name}rstd", name=f"{name}rstd")
    sc = P.tile([128, MT], F32, tag=f"{name}sc", name=f"{name}sc")
    sh = P.tile([128, MT], F32, tag=f"{name}sh", name=f"{name}sh")
    with nc.named_scope(f"{name}_scales"):
        nc.vector.tensor_scalar_mul(ms[:], gs[:], 1.0 / COUNT)
        nc.vector.tensor_mul(var[:], ms[:, 0:MT], ms[:, 0:MT])      # mean^2
        nc.vector.tensor_sub(var[:], ms[:, MT:2 * MT], var[:])      # E[x^2]-m^2
        nc.scalar.activation(std[:], var[:], AF.Sqrt, bias=epsc[:, 0:1])
        nc.vector.reciprocal(rstd[:], std[:])
        nc.vector.tensor_mul(sc[:], gamma[:], rstd[:])              # g * rstd
        nc.vector.tensor_mul(sh[:], ms[:, 0:MT], sc[:])             # m * sc
        nc.vector.tensor_sub(sh[:], beta[:], sh[:])                 # b - m*sc
    return sc, sh


# ----------------------------------------------------------------------------
# Host side: prep per-core inputs, run, assemble outputs
# ----------------------------------------------------------------------------

_NC = None


def _get_nc():
    global _NC
    if _NC is None:
        _NC = _build()
    return _NC


def _bf16(a):
    return np.asarray(a, dtype=np.float32).astype(ml_dtypes.bfloat16)


def _prep(x, Wa, ga, ba, Wb, bb, W1, g1, b1, W4, b4):
    x = np.asarray(x, np.float32)
    # weight transforms (shared across cores)
    wa_t = np.ascontiguousarray(
        _bf16(Wa).reshape(MT, 128, CT_X, 128, 3, 3)
        .transpose(0, 2, 4, 5, 3, 1)                # [m, ct, ki, kj, cp, ml]
        .reshape(MT, 4, CHUNK, 128, 128)
        .transpose(0, 1, 3, 2, 4))                  # [m, kb, cp, s, ml]
    w1_t = np.ascontiguousarray(
        _bf16(W1).reshape(MT, 128, CT_X + CT_A, 128, 3, 3)
        .transpose(0, 2, 4, 5, 3, 1)
        .reshape(MT, 5, CHUNK, 128, 128)
        .transpose(0, 1, 3, 2, 4))
    wb_t = np.ascontiguousarray(
        _bf16(Wb[:, :, 0, 0]).reshape(MT, 128, CT_A, 128).transpose(3, 2, 0, 1))
    w4_t = np.ascontiguousarray(
        _bf16(W4[:, :, 0, 0]).reshape(NCLS, CT_A, 128).transpose(2, 1, 0))

    def cols(v):
        return np.ascontiguousarray(
            np.asarray(v, np.float32).reshape(MT, 128).T)

    b4_c = np.zeros((128, 1), np.float32)
    b4_c[:NCLS, 0] = np.asarray(b4, np.float32)

    xbf = _bf16(x).reshape(B, CT_X, 128, H, W)

    in_maps = []
    for c in range(NCORES):
        b, half = divmod(c, 2)
        r0 = OWN * half
        lo = r0 - 1
        xp = np.zeros((CT_X, 128, XR, XC), ml_dtypes.bfloat16)
        glo, ghi = max(lo, 0), min(lo + XR, H)
        xp[:, :, glo - lo: ghi - lo, 1: 1 + W] = xbf[b, :, :, glo:ghi, :]
        msk = np.ones((128, OBR, W), ml_dtypes.bfloat16)
        if half == 0:
            msk[:, 0, :] = 0        # obp row 0 is global row -1
        else:
            msk[:, OBR - 1, :] = 0  # obp row 33 is global row 64
        in_maps.append({
            "xp": xp, "mask": msk, "wa": wa_t, "w1": w1_t, "wb": wb_t,
            "w4": w4_t, "ga": cols(ga), "ba": cols(ba), "bb": cols(bb),
            "g1": cols(g1), "b1": cols(b1), "b4": b4_c,
        })
    return in_maps


def run(inputs, trace=False):
    nc = _get_nc()
    in_maps = _prep(**inputs)
    res = run_bass_kernel_spmd(
        nc, in_maps, core_ids=list(range(NCORES)), trace=trace)
    out = np.empty((B, CMID, H, W), np.float32)
    out_final = np.empty((B, NCLS, H, W), np.float32)
    for c in range(NCORES):
        b, half = divmod(c, 2)
        r0 = OWN * half
        o = res.results[c]["out_sh"].reshape(CMID, OWN, W)
        out[b, :, r0: r0 + OWN, :] = o
        of = res.results[c]["ofin_sh"].reshape(NCLS, OWN, W)
        out_final[b, :, r0: r0 + OWN, :] = of
    return (out, out_final), res


def kernel(**inputs):
    (out, out_final), _ = run(inputs, trace=False)
    return out, out_final
